# revision 22
# baseline (speedup 1.0000x reference)
"""Trainium2 Bass kernel for nn_Microscope (scatter_memory).

Data-parallel over batch: core c owns slab b=c (H=128, W=128, D=64).
Fully device-side pipeline per core:
  1. Extraction: locs*(iota+1); per-(row, 512-chunk) top-8 -> flat indices.
  2. Per-w-bucket compaction (sparse_gather) into 3 batches of 128 slots.
  3. Value fetch: dma_gather of 64-elem d-lines + one-hot extract.
  4. Placed Gaussian profile evaluation (LH/LY/LZ) + normalization.
  5. rhs = LY (x) LZ outer product; psum[h,(j,k)] += LH.T @ rhs per bucket.
  6. Crop-add psum into OUT (128h, w*64+d); DMA out.
"""
import sys
for _p in ('/opt/trn_rl_repo',):
    if _p not in sys.path:
        sys.path.insert(0, _p)
import math
import numpy as np

import concourse.bass as bass
import concourse.bacc as bacc
import concourse.mybir as mybir
import concourse.tile as tile

F32 = mybir.dt.float32
F16 = mybir.dt.float16
I32 = mybir.dt.int32
U32 = mybir.dt.uint32
I16 = mybir.dt.int16
AF = mybir.ActivationFunctionType
OP = mybir.AluOpType

H, W, D = 128, 128, 64
PSF, R_ = 9, 4
SIG2x2 = 4.5
NBUCK, NBPB = 8, 3
NB = NBUCK * NBPB
WJ, WK, WH = 24, 72, 128
MASK_BIG = 1000.0
NCHUNK = 16
IN_NAMES_V1 = ["locs", "xs", "ys", "zs", "ivs", "scb"]
N_CORES = 8


def eval_axis(nc, pool, posl, off, Wn, name, io3m, masked=True, out_dtype=F32):
    """Placed masked Gaussian: out[e, (b, x)] = exp(-((x - posl) - off)^2/4.5)
    masked to |x - posl| <= 4.  posl/off: (128, NB) f32.  out: (128, NB*Wn).
    io3m: master iota view [128, NB, 128] (const)."""
    t = pool.tile([128, NB * Wn], F32, name=f"t_{name}", tag="w1")
    pos_b = posl[:].rearrange("p b -> p b ()").broadcast_to([128, NB, Wn])
    off_b = off[:].rearrange("p b -> p b ()").broadcast_to([128, NB, Wn])
    io3 = io3m[:, :, 0:Wn]
    t3 = t[:].rearrange("p (b x) -> p b x", x=Wn)
    nc.vector.tensor_tensor(out=t3, in0=io3, in1=pos_b, op=OP.subtract)
    mq = pool.tile([128, NB * Wn], F32, name=f"mq_{name}", tag="ev_mq")
    if masked:
        nc.scalar.activation(mq[:], t[:], AF.Square)
        mk = pool.tile([128, NB * Wn], F32, name=f"mk_{name}", tag="ev_mk")
        nc.vector.tensor_scalar(out=mk[:], in0=mq[:], scalar1=16.2, scalar2=None,
                                op0=OP.is_gt)
    nc.vector.tensor_tensor(out=t3, in0=t3, in1=off_b, op=OP.subtract)
    nc.scalar.activation(mq[:], t[:], AF.Square, scale=1.0 / math.sqrt(SIG2x2))
    if masked:
        nc.vector.scalar_tensor_tensor(out=mk[:], in0=mk[:], scalar=MASK_BIG,
                                       in1=mq[:], op0=OP.mult, op1=OP.add)
        src = mk
    else:
        src = mq
    pl = pool.tile([128, NB * Wn], out_dtype, name=f"pl_{name}", tag=f"pl_{name}")
    nc.scalar.activation(pl[:], src[:], AF.Exp, scale=-1.0)
    return pl


def build_consts(nc, cpool):
    """Constants shared by all reps (Pool iota is very slow: hoist)."""
    io = cpool.tile([128, NB * WH], F32, name="io_c", tag="io_c")
    nc.gpsimd.iota(io[:], pattern=[[0, NB], [1, WH]], base=0,
                   channel_multiplier=0, allow_small_or_imprecise_dtypes=True)
    basei = cpool.tile([128, 128], F32, name="basei_c", tag="basei_c")
    nc.gpsimd.iota(basei[:], pattern=[[512, NCHUNK], [0, 8]], base=-1,
                   channel_multiplier=8192,
                   allow_small_or_imprecise_dtypes=True)
    SP = cpool.tile([16, 192], F32, name="SP_c", tag="SP_c")
    nc.gpsimd.iota(SP[:], pattern=[[0, 8], [16, 24]], base=0,
                   channel_multiplier=1, allow_small_or_imprecise_dtypes=True)
    woff = cpool.tile([128, NB], F32, name="woff_c", tag="woff_c")
    nc.gpsimd.iota(woff[:], pattern=[[16, NBUCK], [0, NBPB]], base=-4,
                   channel_multiplier=0, allow_small_or_imprecise_dtypes=True)
    four = cpool.tile([128, NB], F32, name="four_c", tag="four_c")
    nc.gpsimd.memset(four[:], 4.0)
    return dict(io=io, basei=basei, SP=SP, woff=woff, four=four)


def extract_tables(nc, tc, pool, dpool, ins, C):
    locs_d, xs_d, ys_d, zs_d, ivs_d, scb_d = ins
    X = mybir.AxisListType.X
    io3m = C["io"][:].rearrange("p (b x) -> p b x", x=WH)
    if True:
        # ---- E: extraction ----
        locs_t = pool.tile([128, 8192], F16, name="locs_t", tag="w1")
        nc.sync.dma_start(out=locs_t[:], in_=locs_d[:])
        MXh = pool.tile([128, 128], F16, name="MXh", tag="MXh")
        for c in range(NCHUNK):
            nc.vector.max(MXh[:, c * 8:(c + 1) * 8],
                          locs_t[:, c * 512:(c + 1) * 512])
        MX = pool.tile([128, 128], F32, name="MX", tag="MX")
        nc.vector.tensor_copy(MX[:], MXh[:])
        # slot math -> flat linear index, -1 for empty slots
        basei = C["basei"]
        vld0 = pool.tile([128, 128], F32, name="vld0", tag="vld0")
        nc.vector.tensor_scalar(out=vld0[:], in0=MX[:], scalar1=0.5, scalar2=None,
                                op0=OP.is_gt)
        nc.vector.tensor_tensor(out=MX[:], in0=MX[:], in1=basei[:], op=OP.add)
        nc.vector.tensor_tensor(out=MX[:], in0=MX[:], in1=vld0[:], op=OP.mult)
        nc.vector.scalar_tensor_tensor(out=MX[:], in0=vld0[:], scalar=1.0,
                                       in1=MX[:], op0=OP.subtract, op1=OP.add)
        # MX now holds flat idx (>=0) or -1

        # bounce 1: (128h,128c) -> SG_IN [16, 8 buckets * 128]
        flat1 = dpool.tile([1, 16384], F32, name="flat1", tag="flat1")
        f1w = flat1[:].rearrange("a (p c) -> (a p) c", p=128)
        w_inst = nc.sync.dma_start(out=f1w, in_=MX[:])
        # read layout: addr = ((h16*16+p2)*8+t)*16+c ; dst (p2, t, c, h16)
        f1r = flat1[:].rearrange("a (h16 p2 t c) -> (a p2) t c h16",
                                 h16=8, p2=16, t=8, c=16)
        SG_IN = pool.tile([16, 1024], F32, name="SG_IN", tag="SG_IN")
        sgin_v = SG_IN[:].rearrange("p (t c h16) -> p t c h16", t=8, c=16, h16=8)
        r_inst = nc.sync.dma_start(out=sgin_v, in_=f1r)

        SG_OUT = pool.tile([16, 192], F32, name="SG_OUT", tag="SG_OUT")
        NF = pool.tile([1, 8], U32, name="NF", tag="NF")
        for t in range(NBUCK):
            nc.gpsimd.sparse_gather(SG_OUT[:, t * 24:(t + 1) * 24],
                                    SG_IN[:, t * 128:(t + 1) * 128],
                                    num_found=NF[:, t:t + 1])
        NF16 = pool.tile([16, 8], U32, name="NF16", tag="NF16")
        nc.gpsimd.partition_broadcast(NF16[:], NF[:], channels=16)
        NF16f = pool.tile([16, 8], F32, name="NF16f", tag="NF16f")
        nc.vector.tensor_copy(NF16f[:], NF16[:])
        SP = C["SP"]
        SV = pool.tile([16, 192], F32, name="SV", tag="SV")
        nf_b = NF16f[:].rearrange("p t -> p t ()").broadcast_to([16, 8, 24])
        nc.vector.tensor_tensor(out=SV[:].rearrange("p (t f) -> p t f", f=24),
                                in0=SP[:].rearrange("p (t f) -> p t f", f=24),
                                in1=nf_b, op=OP.is_lt)
        # masked flat idx on wrapped layout (tail garbage -> -1 -> 0 after max)
        nc.vector.tensor_tensor(out=SG_OUT[:], in0=SG_OUT[:], in1=SV[:],
                                op=OP.mult)
        nc.vector.scalar_tensor_tensor(out=SG_OUT[:], in0=SV[:], scalar=1.0,
                                       in1=SG_OUT[:], op0=OP.subtract, op1=OP.add)
        nc.vector.tensor_scalar(out=SG_OUT[:], in0=SG_OUT[:], scalar1=0.0,
                                scalar2=None, op0=OP.max)
        # idx16 for dma_gather: hw128 = flatidx >> 6 (int)
        LI = pool.tile([16, 192], I32, name="LI", tag="LI")
        nc.vector.tensor_copy(LI[:], SG_OUT[:])
        HWI = pool.tile([16, 192], I32, name="HWI", tag="HWI")
        nc.vector.tensor_scalar(out=HWI[:], in0=LI[:], scalar1=6, scalar2=None,
                                op0=OP.arith_shift_right)
        IDX16s = pool.tile([16, 192], I16, name="IDX16s", tag="IDX16s")
        nc.vector.tensor_copy(IDX16s[:], HWI[:])
        IDX = pool.tile([128, 192], I16, name="IDX", tag="IDX")
        for g in range(8):
            nc.sync.dma_start(out=IDX[g * 16:(g + 1) * 16, :], in_=IDX16s[:])

        # bounce 2: wrapped [16,192] -> batch layout (128, 24): lidx + svalid
        flat2 = dpool.tile([1, 6144], F32, name="flat2", tag="flat2")
        f2w_l = flat2[:, 0:3072].rearrange("a (t f p2) -> (a p2) t f",
                                           t=8, f=24, p2=16)
        nc.sync.dma_start(out=f2w_l, in_=SG_OUT[:].rearrange(
            "p (t f) -> p t f", t=8, f=24))
        f2w_v = flat2[:, 3072:6144].rearrange("a (t f p2) -> (a p2) t f",
                                              t=8, f=24, p2=16)
        nc.sync.dma_start(out=f2w_v, in_=SV[:].rearrange(
            "p (t f) -> p t f", t=8, f=24))
        LT = pool.tile([128, NB], F32, name="LT", tag="LT")
        nc.sync.dma_start(out=LT[:], in_=flat2[:, 0:3072].rearrange(
            "a (t j p) -> (a p) (t j)", t=8, j=3, p=128))
        VT = pool.tile([128, NB], F32, name="VT", tag="VT")
        nc.sync.dma_start(out=VT[:], in_=flat2[:, 3072:6144].rearrange(
            "a (t j p) -> (a p) (t j)", t=8, j=3, p=128))

        # decode positions (int ops)
        LTi = pool.tile([128, NB], I32, name="LTi", tag="LTi")
        nc.vector.tensor_copy(LTi[:], LT[:])
        phi = pool.tile([128, NB], I32, name="phi", tag="phi")
        nc.vector.tensor_scalar(out=phi[:], in0=LTi[:], scalar1=13, scalar2=None,
                                op0=OP.arith_shift_right)
        pwi = pool.tile([128, NB], I32, name="pwi", tag="pwi")
        nc.vector.tensor_scalar(out=pwi[:], in0=LTi[:], scalar1=6, scalar2=127,
                                op0=OP.arith_shift_right, op1=OP.bitwise_and)
        pdi = pool.tile([128, NB], I32, name="pdi", tag="pdi")
        nc.vector.tensor_scalar(out=pdi[:], in0=LTi[:], scalar1=63, scalar2=None,
                                op0=OP.bitwise_and)
        ph = pool.tile([128, NB], F32, name="ph_t", tag="ph_t")
        nc.vector.tensor_copy(ph[:], phi[:])
        pw = pool.tile([128, NB], F32, name="pw_t", tag="pw_t")
        nc.vector.tensor_copy(pw[:], pwi[:])
        pd = pool.tile([128, NB], F32, name="pd_t", tag="pd_t")
        nc.vector.tensor_copy(pd[:], pdi[:])

        woff = C["woff"]
        pwl = pool.tile([128, NB], F32, name="pwl", tag="pwl")
        nc.vector.tensor_tensor(out=pwl[:], in0=pw[:], in1=woff[:], op=OP.subtract)
        nc.vector.tensor_scalar(out=pwl[:], in0=pwl[:], scalar1=4.0, scalar2=None,
                                op0=OP.max)
        pdl = pool.tile([128, NB], F32, name="pdl", tag="pdl")
        nc.vector.tensor_scalar(out=pdl[:], in0=pd[:], scalar1=4.0, scalar2=None,
                                op0=OP.add)

        # value gathers: rows of 64 from each slab; extract element d via onehot
        OH = pool.tile([128, NB * 64], F32, name="OH", tag="OH")
        pd_b = pd[:].rearrange("p b -> p b ()").broadcast_to([128, NB, 64])
        nc.vector.tensor_tensor(out=OH[:].rearrange("p (b k) -> p b k", k=64),
                                in0=io3m[:, :, 0:64],
                                in1=pd_b, op=OP.is_equal)
        vals = {}
        GV = pool.tile([128, NB * 64], F32, name="GV", tag="GV")
        for nm, slab in (("vx", xs_d), ("vy", ys_d), ("vz", zs_d), ("vi", ivs_d)):
            for cg in range(3):
                nc.gpsimd.dma_gather(
                    out_ap=GV[:, cg * 512:(cg + 1) * 512].rearrange(
                        "p (b k) -> p b k", k=64),
                    in_ap=slab[:].rearrange("p (w k) -> (p w) k", k=64),
                    idxs_ap=IDX[:, cg * 64:(cg + 1) * 64],
                    num_idxs=1024, num_idxs_reg=1024, elem_size=64)
            nc.gpsimd.tensor_tensor(out=GV[:], in0=GV[:], in1=OH[:], op=OP.mult)
            v = pool.tile([128, NB], F32, name=f"val_{nm}", tag=f"val_{nm}")
            nc.vector.tensor_reduce(out=v[:], axis=X,
                                    in_=GV[:].rearrange("p (b k) -> p b k", k=64),
                                    op=OP.add)
            vals[nm] = v
        scb_t = pool.tile([128, 1], F32, name="scb_t", tag="scb_t")
        nc.sync.dma_start(out=scb_t[:], in_=scb_d[:])
        sc_b = scb_t[:].broadcast_to([128, NB])
        nc.vector.tensor_tensor(out=vals["vi"][:], in0=vals["vi"][:], in1=sc_b,
                                op=OP.mult)
        nc.vector.tensor_tensor(out=vals["vi"][:], in0=vals["vi"][:], in1=VT[:],
                                op=OP.mult)

        return dict(ph=ph, pwl=pwl, pdl=pdl, vx=vals["vx"], vy=vals["vy"],
                    vz=vals["vz"], vi=vals["vi"], VT=VT, LT=LT)


def body_v1(tc, outs, ins, C):
    nc = tc.nc
    out_d = outs[0]
    io3m = C["io"][:].rearrange("p (b x) -> p b x", x=WH)
    X = mybir.AxisListType.X
    with (tc.tile_pool(name="pool", bufs=1) as pool,
          tc.tile_pool(name="rhsp", bufs=3) as rhsp,
          tc.tile_pool(name="psump", bufs=2, space="PSUM") as psump,
          tc.tile_pool(name="dram", bufs=1, space="DRAM") as dpool):
        tb_ = extract_tables(nc, tc, pool, dpool, ins, C)
        ph, pwl, pdl = tb_["ph"], tb_["pwl"], tb_["pdl"]
        vals = {k: tb_[k] for k in ("vx", "vy", "vz", "vi")}
        # ---- profiles / normalization / matmul (same as v0) ----
        LH = eval_axis(nc, pool, ph, vals["vx"], WH, "x", io3m)
        LY = eval_axis(nc, pool, pwl, vals["vy"], WJ, "y", io3m)
        LZ = eval_axis(nc, pool, pdl, vals["vz"], WK, "z", io3m)
        CX = eval_axis(nc, pool, C["four"], vals["vx"], PSF, "cx", io3m,
                       masked=False)

        s_x = pool.tile([128, NB], F32, name="sx", tag="sx")
        s_y = pool.tile([128, NB], F32, name="sy", tag="sy")
        s_z = pool.tile([128, NB], F32, name="sz", tag="sz")
        nc.vector.tensor_reduce(out=s_x[:], axis=X,
                                in_=CX[:].rearrange("p (b x) -> p b x", x=PSF),
                                op=OP.add)
        nc.vector.tensor_reduce(out=s_y[:], axis=X,
                                in_=LY[:].rearrange("p (b x) -> p b x", x=WJ),
                                op=OP.add)
        nc.vector.tensor_reduce(out=s_z[:], axis=X,
                                in_=LZ[:].rearrange("p (b x) -> p b x", x=WK),
                                op=OP.add)
        nc.vector.tensor_tensor(out=s_x[:], in0=s_x[:], in1=s_y[:], op=OP.mult)
        nc.vector.tensor_tensor(out=s_x[:], in0=s_x[:], in1=s_z[:], op=OP.mult)
        nc.vector.reciprocal(s_y[:], s_x[:])
        nc.vector.tensor_tensor(out=s_y[:], in0=s_y[:], in1=vals["vi"][:],
                                op=OP.mult)
        amp_b = s_y[:].rearrange("p b -> p b ()").broadcast_to([128, NB, WH])
        LH3 = LH[:].rearrange("p (b x) -> p b x", x=WH)
        nc.vector.tensor_tensor(out=LH3, in0=LH3, in1=amp_b, op=OP.mult)

        out_t = pool.tile([128, W * D], F32, name="out_t", tag="out_t")
        nc.gpsimd.memset(out_t[:], 0)

        for tb in range(NBUCK):
            ps = psump.tile([128, WJ * WK], F32, name=f"ps{tb}", tag="ps")
            for j in range(NBPB):
                b = tb * NBPB + j
                rhs = rhsp.tile([128, WJ * WK], F32, name=f"rhs{b}", tag="rhs")
                rhs3 = rhs[:].rearrange("p (j k) -> p j k", k=WK)
                ly_b = LY[:, bass.ts(b, WJ)].rearrange(
                    "p j -> p j ()").broadcast_to([128, WJ, WK])
                lz_b = LZ[:, bass.ts(b, WK)].rearrange(
                    "p k -> p () k").broadcast_to([128, WJ, WK])
                nc.gpsimd.tensor_tensor(out=rhs3, in0=ly_b, in1=lz_b, op=OP.mult)
                for c0 in range(0, WJ * WK, 512):
                    c1 = min(c0 + 512, WJ * WK)
                    nc.tensor.matmul(ps[:, c0:c1], lhsT=LH[:, bass.ts(b, WH)],
                                     rhs=rhs[:, c0:c1],
                                     start=(j == 0), stop=(j == NBPB - 1))
            jlo = max(0, 4 - 16 * tb)
            jhi = min(WJ, 4 + W - 16 * tb)
            nj = jhi - jlo
            w0 = 16 * tb - 4 + jlo
            out_sl = out_t[:, w0 * D:(w0 + nj) * D].rearrange(
                "p (j k) -> p j k", k=D)
            ps_sl = ps[:].rearrange("p (j k) -> p j k", k=WK)[:, jlo:jhi,
                                                             R_:R_ + D]
            nc.vector.tensor_tensor(out=out_sl, in0=out_sl, in1=ps_sl, op=OP.add)

        nc.sync.dma_start(out=out_d[:], in_=out_t[:])






def build_nc(repeats=1):
    nc = bacc.Bacc("TRN2", target_bir_lowering=False, debug=False,
                   num_devices=N_CORES)
    ins = []
    for nm in IN_NAMES_V1:
        shape = [128, 1] if nm == "scb" else [128, 8192]
        dt = F16 if nm == "locs" else F32
        ins.append(nc.dram_tensor(nm, shape, dt, kind="ExternalInput").ap())
    out_d = nc.dram_tensor("out", [128, W * D], F32, kind="ExternalOutput").ap()
    with tile.TileContext(nc) as tc:
        with tc.tile_pool(name="cpool", bufs=1) as cpool:
            C = build_consts(tc.nc, cpool)
            for _rep in range(repeats):
                body_v1(tc, [out_d], ins, C)
    nc.compile()
    return nc


class _SpmdRunner:
    def __init__(self, nc, n_cores=N_CORES):
        import jax
        import jax.numpy as jnp
        from jax.sharding import Mesh, PartitionSpec
        from jax.experimental.shard_map import shard_map
        from concourse import bass2jax
        from concourse.bass2jax import _bass_exec_p, partition_id_tensor
        bass2jax.install_neuronx_cc_hook()
        self.jax, self.jnp = jax, jnp
        self.n_cores = n_cores
        in_names, out_names, out_avals, zero_outs = [], [], [], []
        pname = nc.partition_id_tensor.name if nc.partition_id_tensor else None
        for alloc in nc.m.functions[0].allocations:
            if not isinstance(alloc, mybir.MemoryLocationSet):
                continue
            name = alloc.memorylocations[0].name
            if alloc.kind == "ExternalInput":
                if name != pname:
                    in_names.append(name)
            elif alloc.kind == "ExternalOutput":
                shape = tuple(alloc.tensor_shape)
                dtype = mybir.dt.np(alloc.dtype)
                out_names.append(name)
                out_avals.append(jax.core.ShapedArray(shape, dtype))
                zero_outs.append(np.zeros(shape, dtype))
        self.in_names, self.out_names = in_names, out_names
        self.out_avals, self.zero_outs = out_avals, zero_outs
        n_params, n_outs = len(in_names), len(out_avals)
        all_in = in_names + out_names + ([pname] if pname else [])

        def _fn(*args):
            operands = list(args)
            if pname is not None:
                operands.append(partition_id_tensor())
            return tuple(_bass_exec_p.bind(
                *operands, out_avals=tuple(out_avals), in_names=tuple(all_in),
                out_names=tuple(out_names), lowering_input_output_aliases=(),
                sim_require_finite=True, sim_require_nnan=True, nc=nc))

        devices = jax.devices()[:n_cores]
        mesh = Mesh(np.asarray(devices), ("core",))
        specs = (PartitionSpec("core"),)
        self.sharded = jax.jit(
            shard_map(_fn, mesh=mesh, in_specs=specs * (n_params + n_outs),
                      out_specs=specs * n_outs),
            donate_argnums=tuple(range(n_params, n_params + n_outs)),
            keep_unused=True)

    def run(self, in_maps):
        concat = [np.concatenate([np.asarray(m[n]) for m in in_maps], axis=0)
                  for n in self.in_names]
        zeros = [self.jnp.zeros((self.n_cores * z.shape[0], *z.shape[1:]),
                                z.dtype) for z in self.zero_outs]
        outs = self.sharded(*concat, *zeros)
        self.jax.block_until_ready(outs)
        return [
            {n: np.asarray(outs[i]).reshape(self.n_cores,
                                            *self.out_avals[i].shape)[c]
             for i, n in enumerate(self.out_names)}
            for c in range(self.n_cores)]


_RUNNER_CACHE = {}


def _get_runner(repeats=1):
    if repeats not in _RUNNER_CACHE:
        _RUNNER_CACHE[repeats] = _SpmdRunner(build_nc(repeats))
    return _RUNNER_CACHE[repeats]


_CHUNK_POS = ((np.arange(8192, dtype=np.float32) % 512) + 1.0)


def _make_in_maps(locs_3d, x_os_3d, y_os_3d, z_os_3d, ints_3d, scale):
    sc = float(np.asarray(scale).reshape(-1)[0])
    scb = np.full((128, 1), 1000.0 * sc, np.float32)
    in_maps = []
    for c in range(N_CORES):
        m = {"scb": scb}
        locs = np.asarray(locs_3d)[c, 0].reshape(128, 8192)
        m["locs"] = (locs * _CHUNK_POS).astype(np.float16)
        for nm, t in (("xs", x_os_3d), ("ys", y_os_3d),
                      ("zs", z_os_3d), ("ivs", ints_3d)):
            m[nm] = np.ascontiguousarray(
                np.asarray(t)[c, 0].reshape(128, 8192).astype(np.float32,
                                                              copy=False))
        in_maps.append(m)
    return in_maps


def kernel(locs_3d, x_os_3d, y_os_3d, z_os_3d, ints_3d, scale):
    runner = _get_runner()
    in_maps = _make_in_maps(locs_3d, x_os_3d, y_os_3d, z_os_3d, ints_3d, scale)
    res = runner.run(in_maps)
    out = np.stack([res[c]["out"].reshape(H, W, D) for c in range(N_CORES)])
    return out[:, None].astype(np.float32)



# revision 24
# speedup vs baseline: 1.0291x; 1.0291x over previous
"""Trainium2 Bass kernel for nn_Microscope (scatter_memory).

Data-parallel over batch: core c owns slab b=c (H=128, W=128, D=64).
Fully device-side pipeline per core:
  1. Extraction: locs*(iota+1); per-(row, 512-chunk) top-8 -> flat indices.
  2. Per-w-bucket compaction (sparse_gather) into 3 batches of 128 slots.
  3. Value fetch: dma_gather of 64-elem d-lines + one-hot extract.
  4. Placed Gaussian profile evaluation (LH/LY/LZ) + normalization.
  5. rhs = LY (x) LZ outer product; psum[h,(j,k)] += LH.T @ rhs per bucket.
  6. Crop-add psum into OUT (128h, w*64+d); DMA out.
"""
import sys
for _p in ('/opt/trn_rl_repo',):
    if _p not in sys.path:
        sys.path.insert(0, _p)
import math
import numpy as np

import concourse.bass as bass
import concourse.bacc as bacc
import concourse.mybir as mybir
import concourse.tile as tile

F32 = mybir.dt.float32
I32 = mybir.dt.int32
U32 = mybir.dt.uint32
I16 = mybir.dt.int16
AF = mybir.ActivationFunctionType
OP = mybir.AluOpType

H, W, D = 128, 128, 64
PSF, R_ = 9, 4
SIG2x2 = 4.5
DERF_SCALE = 1.0 / math.sqrt(SIG2x2)      # exp(-(s*t)^2) = exp(-t^2/4.5)
CPRIME = (2.0 / math.sqrt(math.pi)) * math.exp(-4.5) * 1.0005
NBUCK, NBPB = 8, 3
NB = NBUCK * NBPB
WJ, WK, WH = 24, 72, 128
MASK_BIG = 1000.0
NCHUNK = 16
IN_NAMES_V1 = ["locs", "xs", "ys", "zs", "ivs", "scb"]
N_CORES = 8


def eval_axis(nc, pool, posl, off, Wn, name, io3m, masked=True, out_dtype=F32):
    """Placed masked Gaussian via one ACT pass:
    out[e,(b,x)] = mask((2/sqrt(pi)) * exp(-((x - posl - off)^2)/4.5)).
    The 2/sqrt(pi) factor cancels in normalization (CX uses it too).
    Mask (g > c)*g cuts |x-posl-off| > 4.5 == ref window (up to off = -0.5
    exactly, measure-zero). posl/off: (128, NB) f32. out: (128, NB*Wn)."""
    po = pool.tile([128, NB], F32, name=f"po_{name}", tag=f"po_{name}")
    nc.vector.tensor_tensor(out=po[:], in0=posl[:], in1=off[:], op=OP.add)
    t = pool.tile([128, NB * Wn], F32, name=f"t_{name}", tag="w1")
    po_b = po[:].rearrange("p b -> p b ()").broadcast_to([128, NB, Wn])
    t3 = t[:].rearrange("p (b x) -> p b x", x=Wn)
    nc.vector.tensor_tensor(out=t3, in0=io3m[:, :, 0:Wn], in1=po_b,
                            op=OP.subtract)
    pl = pool.tile([128, NB * Wn], out_dtype, name=f"pl_{name}",
                   tag=f"pl_{name}")
    nc.scalar.activation(pl[:], t[:], AF.Derivative_Erf, scale=DERF_SCALE)
    if masked:
        nc.vector.scalar_tensor_tensor(out=pl[:], in0=pl[:], scalar=CPRIME,
                                       in1=pl[:], op0=OP.is_gt, op1=OP.mult)
    return pl


def build_consts(nc, cpool):
    """Constants shared by all reps (Pool iota is very slow: hoist)."""
    iotaF = cpool.tile([128, 8192], F32, name="iotaF_c", tag="iotaF_c")
    nc.gpsimd.iota(iotaF[:], pattern=[[0, NCHUNK], [1, 512]], base=1,
                   channel_multiplier=0, allow_small_or_imprecise_dtypes=True)
    io = cpool.tile([128, NB * WH], F32, name="io_c", tag="io_c")
    nc.gpsimd.iota(io[:], pattern=[[0, NB], [1, WH]], base=0,
                   channel_multiplier=0, allow_small_or_imprecise_dtypes=True)
    basei = cpool.tile([128, 128], F32, name="basei_c", tag="basei_c")
    nc.gpsimd.iota(basei[:], pattern=[[512, NCHUNK], [0, 8]], base=-1,
                   channel_multiplier=8192,
                   allow_small_or_imprecise_dtypes=True)
    SP = cpool.tile([16, 192], F32, name="SP_c", tag="SP_c")
    nc.gpsimd.iota(SP[:], pattern=[[0, 8], [16, 24]], base=0,
                   channel_multiplier=1, allow_small_or_imprecise_dtypes=True)
    woff = cpool.tile([128, NB], F32, name="woff_c", tag="woff_c")
    nc.gpsimd.iota(woff[:], pattern=[[16, NBUCK], [0, NBPB]], base=-4,
                   channel_multiplier=0, allow_small_or_imprecise_dtypes=True)
    four = cpool.tile([128, NB], F32, name="four_c", tag="four_c")
    nc.gpsimd.memset(four[:], 4.0)
    return dict(iotaF=iotaF, io=io, basei=basei, SP=SP, woff=woff, four=four)


def extract_tables(nc, tc, pool, dpool, ins, C):
    locs_d, xs_d, ys_d, zs_d, ivs_d, scb_d = ins
    X = mybir.AxisListType.X
    io3m = C["io"][:].rearrange("p (b x) -> p b x", x=WH)
    if True:
        # ---- E: extraction ----
        locs_t = pool.tile([128, 8192], F32, name="locs_t", tag="w1")
        nc.sync.dma_start(out=locs_t[:], in_=locs_d[:])
        nc.vector.tensor_tensor(out=locs_t[:], in0=locs_t[:],
                                in1=C["iotaF"][:], op=OP.mult)
        MX = pool.tile([128, 128], F32, name="MX", tag="MX")
        for c in range(NCHUNK):
            nc.vector.max(MX[:, c * 8:(c + 1) * 8], locs_t[:, c * 512:(c + 1) * 512])
        # slot math -> flat linear index, -1 for empty slots
        basei = C["basei"]
        vld0 = pool.tile([128, 128], F32, name="vld0", tag="vld0")
        nc.vector.tensor_scalar(out=vld0[:], in0=MX[:], scalar1=0.5, scalar2=None,
                                op0=OP.is_gt)
        nc.vector.tensor_tensor(out=MX[:], in0=MX[:], in1=basei[:], op=OP.add)
        nc.vector.tensor_tensor(out=MX[:], in0=MX[:], in1=vld0[:], op=OP.mult)
        nc.vector.scalar_tensor_tensor(out=MX[:], in0=vld0[:], scalar=1.0,
                                       in1=MX[:], op0=OP.subtract, op1=OP.add)
        # MX now holds flat idx (>=0) or -1

        # bounce 1: (128h,128c) -> SG_IN [16, 8 buckets * 128]
        flat1 = dpool.tile([1, 16384], F32, name="flat1", tag="flat1")
        f1w = flat1[:].rearrange("a (p c) -> (a p) c", p=128)
        w_inst = nc.sync.dma_start(out=f1w, in_=MX[:])
        # read layout: addr = ((h16*16+p2)*8+t)*16+c ; dst (p2, t, c, h16)
        f1r = flat1[:].rearrange("a (h16 p2 t c) -> (a p2) t c h16",
                                 h16=8, p2=16, t=8, c=16)
        SG_IN = pool.tile([16, 1024], F32, name="SG_IN", tag="SG_IN")
        sgin_v = SG_IN[:].rearrange("p (t c h16) -> p t c h16", t=8, c=16, h16=8)
        r_inst = nc.sync.dma_start(out=sgin_v, in_=f1r)

        SG_OUT = pool.tile([16, 192], F32, name="SG_OUT", tag="SG_OUT")
        NF = pool.tile([1, 8], U32, name="NF", tag="NF")
        for t in range(NBUCK):
            nc.gpsimd.sparse_gather(SG_OUT[:, t * 24:(t + 1) * 24],
                                    SG_IN[:, t * 128:(t + 1) * 128],
                                    num_found=NF[:, t:t + 1])
        NF16 = pool.tile([16, 8], U32, name="NF16", tag="NF16")
        nc.gpsimd.partition_broadcast(NF16[:], NF[:], channels=16)
        NF16f = pool.tile([16, 8], F32, name="NF16f", tag="NF16f")
        nc.vector.tensor_copy(NF16f[:], NF16[:])
        SP = C["SP"]
        SV = pool.tile([16, 192], F32, name="SV", tag="SV")
        nf_b = NF16f[:].rearrange("p t -> p t ()").broadcast_to([16, 8, 24])
        nc.vector.tensor_tensor(out=SV[:].rearrange("p (t f) -> p t f", f=24),
                                in0=SP[:].rearrange("p (t f) -> p t f", f=24),
                                in1=nf_b, op=OP.is_lt)
        # masked flat idx on wrapped layout (tail garbage -> -1 -> 0 after max)
        nc.vector.tensor_tensor(out=SG_OUT[:], in0=SG_OUT[:], in1=SV[:],
                                op=OP.mult)
        nc.vector.scalar_tensor_tensor(out=SG_OUT[:], in0=SV[:], scalar=1.0,
                                       in1=SG_OUT[:], op0=OP.subtract, op1=OP.add)
        nc.vector.tensor_scalar(out=SG_OUT[:], in0=SG_OUT[:], scalar1=0.0,
                                scalar2=None, op0=OP.max)
        # idx16 for dma_gather: hw128 = flatidx >> 6 (int)
        LI = pool.tile([16, 192], I32, name="LI", tag="LI")
        nc.vector.tensor_copy(LI[:], SG_OUT[:])
        HWI = pool.tile([16, 192], I32, name="HWI", tag="HWI")
        nc.vector.tensor_scalar(out=HWI[:], in0=LI[:], scalar1=6, scalar2=None,
                                op0=OP.arith_shift_right)
        IDX16s = pool.tile([16, 192], I16, name="IDX16s", tag="IDX16s")
        nc.vector.tensor_copy(IDX16s[:], HWI[:])
        IDX = pool.tile([128, 192], I16, name="IDX", tag="IDX")
        for g in range(8):
            nc.sync.dma_start(out=IDX[g * 16:(g + 1) * 16, :], in_=IDX16s[:])

        # bounce 2: wrapped [16,192] -> batch layout (128, 24): lidx + svalid
        flat2 = dpool.tile([1, 6144], F32, name="flat2", tag="flat2")
        f2w_l = flat2[:, 0:3072].rearrange("a (t f p2) -> (a p2) t f",
                                           t=8, f=24, p2=16)
        nc.sync.dma_start(out=f2w_l, in_=SG_OUT[:].rearrange(
            "p (t f) -> p t f", t=8, f=24))
        f2w_v = flat2[:, 3072:6144].rearrange("a (t f p2) -> (a p2) t f",
                                              t=8, f=24, p2=16)
        nc.sync.dma_start(out=f2w_v, in_=SV[:].rearrange(
            "p (t f) -> p t f", t=8, f=24))
        LT = pool.tile([128, NB], F32, name="LT", tag="LT")
        nc.sync.dma_start(out=LT[:], in_=flat2[:, 0:3072].rearrange(
            "a (t j p) -> (a p) (t j)", t=8, j=3, p=128))
        VT = pool.tile([128, NB], F32, name="VT", tag="VT")
        nc.sync.dma_start(out=VT[:], in_=flat2[:, 3072:6144].rearrange(
            "a (t j p) -> (a p) (t j)", t=8, j=3, p=128))

        # decode positions (int ops)
        LTi = pool.tile([128, NB], I32, name="LTi", tag="LTi")
        nc.vector.tensor_copy(LTi[:], LT[:])
        phi = pool.tile([128, NB], I32, name="phi", tag="phi")
        nc.vector.tensor_scalar(out=phi[:], in0=LTi[:], scalar1=13, scalar2=None,
                                op0=OP.arith_shift_right)
        pwi = pool.tile([128, NB], I32, name="pwi", tag="pwi")
        nc.vector.tensor_scalar(out=pwi[:], in0=LTi[:], scalar1=6, scalar2=127,
                                op0=OP.arith_shift_right, op1=OP.bitwise_and)
        pdi = pool.tile([128, NB], I32, name="pdi", tag="pdi")
        nc.vector.tensor_scalar(out=pdi[:], in0=LTi[:], scalar1=63, scalar2=None,
                                op0=OP.bitwise_and)
        ph = pool.tile([128, NB], F32, name="ph_t", tag="ph_t")
        nc.vector.tensor_copy(ph[:], phi[:])
        pw = pool.tile([128, NB], F32, name="pw_t", tag="pw_t")
        nc.vector.tensor_copy(pw[:], pwi[:])
        pd = pool.tile([128, NB], F32, name="pd_t", tag="pd_t")
        nc.vector.tensor_copy(pd[:], pdi[:])

        woff = C["woff"]
        pwl = pool.tile([128, NB], F32, name="pwl", tag="pwl")
        nc.vector.tensor_tensor(out=pwl[:], in0=pw[:], in1=woff[:], op=OP.subtract)
        nc.vector.tensor_scalar(out=pwl[:], in0=pwl[:], scalar1=4.0, scalar2=None,
                                op0=OP.max)
        pdl = pool.tile([128, NB], F32, name="pdl", tag="pdl")
        nc.vector.tensor_scalar(out=pdl[:], in0=pd[:], scalar1=4.0, scalar2=None,
                                op0=OP.add)

        # value gathers: rows of 64 from each slab; extract element d via onehot
        OH = pool.tile([128, NB * 64], F32, name="OH", tag="OH")
        pd_b = pd[:].rearrange("p b -> p b ()").broadcast_to([128, NB, 64])
        nc.vector.tensor_tensor(out=OH[:].rearrange("p (b k) -> p b k", k=64),
                                in0=io3m[:, :, 0:64],
                                in1=pd_b, op=OP.is_equal)
        vals = {}
        GV = pool.tile([128, NB * 64], F32, name="GV", tag="GV")
        for nm, slab in (("vx", xs_d), ("vy", ys_d), ("vz", zs_d), ("vi", ivs_d)):
            for cg in range(3):
                nc.gpsimd.dma_gather(
                    out_ap=GV[:, cg * 512:(cg + 1) * 512].rearrange(
                        "p (b k) -> p b k", k=64),
                    in_ap=slab[:].rearrange("p (w k) -> (p w) k", k=64),
                    idxs_ap=IDX[:, cg * 64:(cg + 1) * 64],
                    num_idxs=1024, num_idxs_reg=1024, elem_size=64)
            nc.gpsimd.tensor_tensor(out=GV[:], in0=GV[:], in1=OH[:], op=OP.mult)
            v = pool.tile([128, NB], F32, name=f"val_{nm}", tag=f"val_{nm}")
            nc.vector.tensor_reduce(out=v[:], axis=X,
                                    in_=GV[:].rearrange("p (b k) -> p b k", k=64),
                                    op=OP.add)
            vals[nm] = v
        scb_t = pool.tile([128, 1], F32, name="scb_t", tag="scb_t")
        nc.sync.dma_start(out=scb_t[:], in_=scb_d[:])
        sc_b = scb_t[:].broadcast_to([128, NB])
        nc.vector.tensor_tensor(out=vals["vi"][:], in0=vals["vi"][:], in1=sc_b,
                                op=OP.mult)
        nc.vector.tensor_tensor(out=vals["vi"][:], in0=vals["vi"][:], in1=VT[:],
                                op=OP.mult)

        return dict(ph=ph, pwl=pwl, pdl=pdl, vx=vals["vx"], vy=vals["vy"],
                    vz=vals["vz"], vi=vals["vi"], VT=VT, LT=LT)


def body_v1(tc, outs, ins, C):
    nc = tc.nc
    out_d = outs[0]
    io3m = C["io"][:].rearrange("p (b x) -> p b x", x=WH)
    X = mybir.AxisListType.X
    with (tc.tile_pool(name="pool", bufs=1) as pool,
          tc.tile_pool(name="rhsp", bufs=3) as rhsp,
          tc.tile_pool(name="psump", bufs=2, space="PSUM") as psump,
          tc.tile_pool(name="dram", bufs=1, space="DRAM") as dpool):
        tb_ = extract_tables(nc, tc, pool, dpool, ins, C)
        ph, pwl, pdl = tb_["ph"], tb_["pwl"], tb_["pdl"]
        vals = {k: tb_[k] for k in ("vx", "vy", "vz", "vi")}
        # ---- profiles / normalization / matmul (same as v0) ----
        LH = eval_axis(nc, pool, ph, vals["vx"], WH, "x", io3m)
        LY = eval_axis(nc, pool, pwl, vals["vy"], WJ, "y", io3m)
        LZ = eval_axis(nc, pool, pdl, vals["vz"], WK, "z", io3m)
        CX = eval_axis(nc, pool, C["four"], vals["vx"], PSF, "cx", io3m,
                       masked=False)

        s_x = pool.tile([128, NB], F32, name="sx", tag="sx")
        s_y = pool.tile([128, NB], F32, name="sy", tag="sy")
        s_z = pool.tile([128, NB], F32, name="sz", tag="sz")
        nc.vector.tensor_reduce(out=s_x[:], axis=X,
                                in_=CX[:].rearrange("p (b x) -> p b x", x=PSF),
                                op=OP.add)
        nc.vector.tensor_reduce(out=s_y[:], axis=X,
                                in_=LY[:].rearrange("p (b x) -> p b x", x=WJ),
                                op=OP.add)
        nc.vector.tensor_reduce(out=s_z[:], axis=X,
                                in_=LZ[:].rearrange("p (b x) -> p b x", x=WK),
                                op=OP.add)
        nc.vector.tensor_tensor(out=s_x[:], in0=s_x[:], in1=s_y[:], op=OP.mult)
        nc.vector.tensor_tensor(out=s_x[:], in0=s_x[:], in1=s_z[:], op=OP.mult)
        nc.vector.reciprocal(s_y[:], s_x[:])
        nc.vector.tensor_tensor(out=s_y[:], in0=s_y[:], in1=vals["vi"][:],
                                op=OP.mult)
        amp_b = s_y[:].rearrange("p b -> p b ()").broadcast_to([128, NB, WH])
        LH3 = LH[:].rearrange("p (b x) -> p b x", x=WH)
        nc.vector.tensor_tensor(out=LH3, in0=LH3, in1=amp_b, op=OP.mult)

        out_t = pool.tile([128, W * D], F32, name="out_t", tag="out_t")
        nc.gpsimd.memset(out_t[:], 0)

        for tb in range(NBUCK):
            ps = psump.tile([128, WJ * WK], F32, name=f"ps{tb}", tag="ps")
            for j in range(NBPB):
                b = tb * NBPB + j
                rhs = rhsp.tile([128, WJ * WK], F32, name=f"rhs{b}", tag="rhs")
                rhs3 = rhs[:].rearrange("p (j k) -> p j k", k=WK)
                ly_b = LY[:, bass.ts(b, WJ)].rearrange(
                    "p j -> p j ()").broadcast_to([128, WJ, WK])
                lz_b = LZ[:, bass.ts(b, WK)].rearrange(
                    "p k -> p () k").broadcast_to([128, WJ, WK])
                nc.gpsimd.tensor_tensor(out=rhs3, in0=ly_b, in1=lz_b, op=OP.mult)
                for c0 in range(0, WJ * WK, 512):
                    c1 = min(c0 + 512, WJ * WK)
                    nc.tensor.matmul(ps[:, c0:c1], lhsT=LH[:, bass.ts(b, WH)],
                                     rhs=rhs[:, c0:c1],
                                     start=(j == 0), stop=(j == NBPB - 1))
            jlo = max(0, 4 - 16 * tb)
            jhi = min(WJ, 4 + W - 16 * tb)
            nj = jhi - jlo
            w0 = 16 * tb - 4 + jlo
            out_sl = out_t[:, w0 * D:(w0 + nj) * D].rearrange(
                "p (j k) -> p j k", k=D)
            ps_sl = ps[:].rearrange("p (j k) -> p j k", k=WK)[:, jlo:jhi,
                                                             R_:R_ + D]
            nc.vector.tensor_tensor(out=out_sl, in0=out_sl, in1=ps_sl, op=OP.add)

        nc.sync.dma_start(out=out_d[:], in_=out_t[:])






def build_nc(repeats=1):
    nc = bacc.Bacc("TRN2", target_bir_lowering=False, debug=False,
                   num_devices=N_CORES)
    ins = []
    for nm in IN_NAMES_V1:
        shape = [128, 1] if nm == "scb" else [128, 8192]
        ins.append(nc.dram_tensor(nm, shape, F32, kind="ExternalInput").ap())
    out_d = nc.dram_tensor("out", [128, W * D], F32, kind="ExternalOutput").ap()
    with tile.TileContext(nc) as tc:
        with tc.tile_pool(name="cpool", bufs=1) as cpool:
            C = build_consts(tc.nc, cpool)
            for _rep in range(repeats):
                body_v1(tc, [out_d], ins, C)
    nc.compile()
    return nc


class _SpmdRunner:
    def __init__(self, nc, n_cores=N_CORES):
        import jax
        import jax.numpy as jnp
        from jax.sharding import Mesh, PartitionSpec
        from jax.experimental.shard_map import shard_map
        from concourse import bass2jax
        from concourse.bass2jax import _bass_exec_p, partition_id_tensor
        bass2jax.install_neuronx_cc_hook()
        self.jax, self.jnp = jax, jnp
        self.n_cores = n_cores
        in_names, out_names, out_avals, zero_outs = [], [], [], []
        pname = nc.partition_id_tensor.name if nc.partition_id_tensor else None
        for alloc in nc.m.functions[0].allocations:
            if not isinstance(alloc, mybir.MemoryLocationSet):
                continue
            name = alloc.memorylocations[0].name
            if alloc.kind == "ExternalInput":
                if name != pname:
                    in_names.append(name)
            elif alloc.kind == "ExternalOutput":
                shape = tuple(alloc.tensor_shape)
                dtype = mybir.dt.np(alloc.dtype)
                out_names.append(name)
                out_avals.append(jax.core.ShapedArray(shape, dtype))
                zero_outs.append(np.zeros(shape, dtype))
        self.in_names, self.out_names = in_names, out_names
        self.out_avals, self.zero_outs = out_avals, zero_outs
        n_params, n_outs = len(in_names), len(out_avals)
        all_in = in_names + out_names + ([pname] if pname else [])

        def _fn(*args):
            operands = list(args)
            if pname is not None:
                operands.append(partition_id_tensor())
            return tuple(_bass_exec_p.bind(
                *operands, out_avals=tuple(out_avals), in_names=tuple(all_in),
                out_names=tuple(out_names), lowering_input_output_aliases=(),
                sim_require_finite=True, sim_require_nnan=True, nc=nc))

        devices = jax.devices()[:n_cores]
        mesh = Mesh(np.asarray(devices), ("core",))
        specs = (PartitionSpec("core"),)
        self.sharded = jax.jit(
            shard_map(_fn, mesh=mesh, in_specs=specs * (n_params + n_outs),
                      out_specs=specs * n_outs),
            donate_argnums=tuple(range(n_params, n_params + n_outs)),
            keep_unused=True)

    def run(self, in_maps):
        concat = [np.concatenate([np.asarray(m[n]) for m in in_maps], axis=0)
                  for n in self.in_names]
        zeros = [self.jnp.zeros((self.n_cores * z.shape[0], *z.shape[1:]),
                                z.dtype) for z in self.zero_outs]
        outs = self.sharded(*concat, *zeros)
        self.jax.block_until_ready(outs)
        return [
            {n: np.asarray(outs[i]).reshape(self.n_cores,
                                            *self.out_avals[i].shape)[c]
             for i, n in enumerate(self.out_names)}
            for c in range(self.n_cores)]


_RUNNER_CACHE = {}


def _get_runner(repeats=1):
    if repeats not in _RUNNER_CACHE:
        _RUNNER_CACHE[repeats] = _SpmdRunner(build_nc(repeats))
    return _RUNNER_CACHE[repeats]


def _make_in_maps(locs_3d, x_os_3d, y_os_3d, z_os_3d, ints_3d, scale):
    sc = float(np.asarray(scale).reshape(-1)[0])
    scb = np.full((128, 1), 1000.0 * sc, np.float32)
    in_maps = []
    for c in range(N_CORES):
        m = {"scb": scb}
        for nm, t in (("locs", locs_3d), ("xs", x_os_3d), ("ys", y_os_3d),
                      ("zs", z_os_3d), ("ivs", ints_3d)):
            m[nm] = np.ascontiguousarray(
                np.asarray(t)[c, 0].reshape(128, 8192).astype(np.float32,
                                                              copy=False))
        in_maps.append(m)
    return in_maps


def kernel(locs_3d, x_os_3d, y_os_3d, z_os_3d, ints_3d, scale):
    runner = _get_runner()
    in_maps = _make_in_maps(locs_3d, x_os_3d, y_os_3d, z_os_3d, ints_3d, scale)
    res = runner.run(in_maps)
    out = np.stack([res[c]["out"].reshape(H, W, D) for c in range(N_CORES)])
    return out[:, None].astype(np.float32)

